# revision 9
# baseline (speedup 1.0000x reference)
"""GATv2Conv message-passing kernel for 8 Trainium2 NeuronCores.

Strategy (receiver-sharded, padded-grid, no collectives):
- Nodes are sorted by in-degree and dealt round-robin to the 8 cores, so each
  core owns ~12.5k receiver nodes with a balanced edge count, and consecutive
  128-node tiles have near-uniform degree (padding ratio ~1.02).
- Each core computes the full sender projection table s_proj = [x|1] @ [Ws;bs]
  on-device into an HBM scratch table (replicated work, fp16, viewed as 50176
  chunks of 2 rows / 256B), and its local receiver projection r_proj into SBUF.
- Receiver tiles are grouped into BATCHES (~16KB/partition of gathered chunk
  data). Per batch, ONE InstDMAGatherAnt (dma_gather) on a rotating SWDGE
  queue fetches all sender chunks: int16 indices biased by -32768 (table base
  offset +32768 chunks) reach all 50176 chunks; the trailing index is a dummy
  with positive biased value so the ucode's trailing-negative trim never eats
  real slots. Each descriptor fetches a 256B chunk (2 rows); a uint8 parity
  plane + copy_predicated picks the right row.
- The per-tile edge math (mish via Exp/Ln/Tanh LUTs, masked softmax with
  max-subtraction, weighted aggregation) is SOFTWARE-PIPELINED in 3 stages
  (s1: select+z+ACT chain, s2: logit reduce+shift+exp, s3: weighted sum) with
  skewed issue order so the in-order DVE/ACT queues overlap across tiles.

Measured hardware facts that shaped this design (axon TRN2 microbenchmarks):
- indirect_dma_start costs ~9.4ns/descriptor (scalar Q7 SWDGE desc-gen, one
  index per partition per instruction) -> 1.6ms for 203k edge descriptors.
- dma_gather desc-gen is 16-lane vectorized on Q7 and queue_num q runs on Q7
  core pair (2q, 2q+1): 1 queue = 6.6ns/desc, 4 queues = 2.7ns/desc,
  independent of 256B vs 512B descriptor size (descriptor-path bound).
- DVE perf modes: TensorTensor is 2x_1p (fp16 2x, dedicated port, never
  locks GpSimd); TensorCopy 4x_2p grabs the shared port GpSimd needs for
  SWDGE; TensorReduce/CopyPredicated run 1x.
- copy_predicated requires a uint8 mask; Tanh LUT exists (fp16 4e-4), Mish
  and Softplus LUTs do not exist in this build.
"""

import numpy as np

import concourse.bass as bass
import concourse.bacc as bacc
import concourse.mybir as mybir
import concourse.tile as tile
from concourse.bass_utils import run_bass_kernel_spmd

F32 = mybir.dt.float32
F16 = mybir.dt.float16
I16 = mybir.dt.int16
U8 = mybir.dt.uint8

N_NODES = 100000
N_EDGES = 1600000
F = 64
H = 4
HD = 16
NC_CORES = 8
BIAS = 32768
DUMMY_CHUNK = 40000   # any chunk in [32768, 50176): positive biased value
BATCH_COLS = 48       # max sum of D_t (+1 dummy col) per gather batch


def _host_prep(x, Ws, bs, Wr, br, aw, ab, senders, receivers):
    """Pure index/layout work: shard nodes+edges, build grid slot arrays."""
    N = x.shape[0]
    deg = np.bincount(receivers, minlength=N)
    order = np.argsort(deg, kind="stable").astype(np.int64)  # rank -> node
    inv_order = np.empty(N, dtype=np.int64)
    inv_order[order] = np.arange(N)

    rows_per_core = -(-N // NC_CORES)          # 12500
    tiles = -(-rows_per_core // 128)           # 98
    rows_pad = tiles * 128                     # 12544

    # per-tile max degree over the 1024-rank window (common across cores)
    d_pad = np.zeros(tiles * 1024, dtype=np.int64)
    d_pad[: N] = deg[order]
    D_t = d_pad.reshape(tiles, 1024).max(axis=1)
    D_t = np.maximum(D_t, 1)
    OFF = np.concatenate([[0], np.cumsum(D_t)]).astype(np.int64)
    S = int(OFF[-1])

    # batches of consecutive tiles with sum(D_t) <= BATCH_COLS - 1
    batches = []          # list of (t_start, t_end)
    bt = 0
    while bt < tiles:
        te = bt
        tot = 0
        while te < tiles and tot + D_t[te] <= BATCH_COLS - 1:
            tot += D_t[te]
            te += 1
        assert te > bt
        batches.append((bt, te))
        bt = te
    nb = len(batches)
    # per-batch: data cols and idx16 col offsets
    BC = [int(sum(D_t[a:b])) for a, b in batches]        # real data cols
    O16 = np.concatenate([[0], np.cumsum([8 * c + 1 for c in BC])]).astype(np.int64)
    SI16 = int(O16[-1])
    # per-tile col offset within its batch
    bat_of = np.empty(tiles, dtype=np.int64)
    ob_t = np.empty(tiles, dtype=np.int64)
    for bi, (a, b) in enumerate(batches):
        o = 0
        for t in range(a, b):
            bat_of[t] = bi
            ob_t[t] = o
            o += D_t[t]

    # edge -> (core, row, k)
    erank = inv_order[receivers]
    e_sort = np.argsort(erank, kind="stable")
    er_sorted = erank[e_sort]
    s_sorted = senders[e_sort]
    grp_start = np.searchsorted(er_sorted, np.arange(N))
    k_all = np.arange(len(er_sorted)) - grp_start[er_sorted]

    core_e = er_sorted % NC_CORES
    row_e = er_sorted // NC_CORES
    t_e = row_e // 128
    p_e = row_e % 128
    col_e = OFF[t_e] + k_all

    # sender node n = g*512 + j*128 + p lands at table row g*512 + p*4 + j
    # (lets phase-1b write 1KB-contiguous runs per partition); chunk = row//2
    g_n = s_sorted // 512
    rem = s_sorted % 512
    j_n = rem // 128
    p_n = rem % 128
    tau = g_n * 512 + p_n * 4 + j_n
    chunk_e = tau // 2
    par_e = (tau % 2).astype(np.uint8)

    mask_arr = np.zeros((NC_CORES, 128, S), dtype=np.float16)
    b0_arr = np.zeros((NC_CORES, 128, S), dtype=np.uint8)
    mask_arr[core_e, p_e, col_e] = 1.0
    b0_arr[core_e, p_e, col_e] = par_e

    # int16 gather index planes, one flat list per batch: position
    # i = c*128 + p over concatenated batch cols, wrapped 16-wide
    chunk_grid = np.full((NC_CORES, 128, S), DUMMY_CHUNK, dtype=np.int64)
    chunk_grid[core_e, p_e, col_e] = chunk_e
    idx16_arr = np.zeros((NC_CORES, 128, SI16), dtype=np.int16)
    for bi, (a, b) in enumerate(batches):
        C = BC[bi]
        ncol = 8 * C + 1
        flat = np.empty((NC_CORES, ncol * 16), dtype=np.int16)
        flat[:, :] = np.int16(DUMMY_CHUNK - BIAS)
        fl = (chunk_grid[:, :, OFF[a]:OFF[a] + C] - BIAS).astype(np.int16)
        flat[:, : 128 * C] = fl.transpose(0, 2, 1).reshape(NC_CORES, -1)
        wrap = flat.reshape(NC_CORES, ncol, 16).transpose(0, 2, 1)
        idx16_arr[:, :, O16[bi]:O16[bi] + ncol] = np.tile(wrap, (1, 8, 1))

    # x^T padded + ones row, shared across cores
    n_grp = -(-N // 512)
    n_tab = n_grp * 512
    n_chunks = n_tab // 2
    xT_aug = np.zeros((F + 1, n_tab), dtype=np.float16)
    xT_aug[:F, :N] = x.T
    xT_aug[F, :] = 1.0

    # per-core local x^T (+ones)
    xlT = np.zeros((NC_CORES, F + 1, rows_pad), dtype=np.float16)
    for c in range(NC_CORES):
        rows = order[c::NC_CORES]          # ranks c, c+8, ... ascending rank
        xlT[c, :F, : len(rows)] = x[rows].T
        xlT[c, F, :] = 1.0

    Wsb = np.concatenate([Ws.reshape(F, F), bs.reshape(1, F)], axis=0).astype(np.float16)
    Wrb = np.concatenate([Wr.reshape(F, F), br.reshape(1, F)], axis=0).astype(np.float16)
    aw_rep = np.tile(np.asarray(aw, np.float32).reshape(1, HD), (1, H)).reshape(1, F)
    awb = np.tile(aw_rep, (128, 1)).astype(np.float32)

    meta = dict(
        D_t=D_t.astype(int).tolist(),
        OFF=OFF.astype(int).tolist(),
        O16=O16.astype(int).tolist(),
        batches=batches,
        BC=BC,
        bat_of=bat_of.astype(int).tolist(),
        ob_t=ob_t.astype(int).tolist(),
        S=S,
        SI16=SI16,
        tiles=tiles,
        rows_pad=rows_pad,
        n_tab=n_tab,
        n_chunks=n_chunks,
        n_grp=n_grp,
        order=order,
        ab=float(np.asarray(ab).reshape(-1)[0]),
    )
    ins = dict(xT=xT_aug, xlT=xlT, Wsb=Wsb, Wrb=Wrb, awb=awb,
               idx16=idx16_arr, b0=b0_arr, mask=mask_arr)
    return ins, meta


VARIANT = "full"  # full | gather_only | compute_only | phase1_only | empty
P2REPS = 1        # repeat phase-2 (timing experiments only)


def _build_program(meta):
    D_t, OFF, O16, S, SI16 = (meta["D_t"], meta["OFF"], meta["O16"],
                              meta["S"], meta["SI16"])
    batches, BC, bat_of, ob_t = (meta["batches"], meta["BC"],
                                 meta["bat_of"], meta["ob_t"])
    tiles, rows_pad, n_tab, n_chunks, n_grp = (
        meta["tiles"], meta["rows_pad"], meta["n_tab"], meta["n_chunks"],
        meta["n_grp"])

    nc = bacc.Bacc(num_swdge_queues=4, dynamic_dma_scratch_size=32768)
    xT = nc.declare_dram_parameter("xT", [F + 1, n_tab], F16, isOutput=False)
    xlT = nc.declare_dram_parameter("xlT", [F + 1, rows_pad], F16, isOutput=False)
    Wsb = nc.declare_dram_parameter("Wsb", [F + 1, F], F16, isOutput=False)
    Wrb = nc.declare_dram_parameter("Wrb", [F + 1, F], F16, isOutput=False)
    awb = nc.declare_dram_parameter("awb", [128, F], F32, isOutput=False)
    idx16p = nc.declare_dram_parameter("idx16", [128, SI16], I16, isOutput=False)
    b0p = nc.declare_dram_parameter("b0", [128, S], U8, isOutput=False)
    maskp = nc.declare_dram_parameter("mask", [128, S], F16, isOutput=False)
    outp = nc.declare_dram_parameter("out", [rows_pad, F], F32, isOutput=True)

    AT = mybir.ActivationFunctionType
    ALU = mybir.AluOpType

    with tile.TileContext(nc) as tc:
        with (
            tc.tile_pool(name="dram", bufs=1, space="DRAM") as dpool,
            tc.tile_pool(name="consts", bufs=1) as cpool,
            tc.tile_pool(name="xload", bufs=3) as xpool,
            tc.tile_pool(name="pch", bufs=3) as pch,
            tc.tile_pool(name="pse", bufs=3) as pse,
            tc.tile_pool(name="pz", bufs=2) as pz,
            tc.tile_pool(name="pet", bufs=2) as pet,
            tc.tile_pool(name="pth", bufs=3) as pth,
            tc.tile_pool(name="pza", bufs=2) as pza,
            tc.tile_pool(name="pwse", bufs=2) as pwse,
            tc.tile_pool(name="small", bufs=4) as spool,
            tc.tile_pool(name="psum", bufs=2, space="PSUM") as ppool,
        ):
            # s_proj table viewed as 2-row (256B) chunks
            table = dpool.tile([n_chunks, 2 * F], F16)

            wsb_sb = cpool.tile([F + 1, F], F16)
            nc.sync.dma_start(out=wsb_sb[:], in_=Wsb[:])
            wrb_sb = cpool.tile([F + 1, F], F16)
            nc.sync.dma_start(out=wrb_sb[:], in_=Wrb[:])
            awb_sb = cpool.tile([128, F], F32)
            nc.sync.dma_start(out=awb_sb[:], in_=awb[:])
            idx_sb = cpool.tile([128, SI16], I16)
            nc.sync.dma_start(out=idx_sb[:], in_=idx16p[:])
            b0_sb = cpool.tile([128, S], U8)
            nc.sync.dma_start(out=b0_sb[:], in_=b0p[:])
            mask_sb = cpool.tile([128, S], F16)
            nc.sync.dma_start(out=mask_sb[:], in_=maskp[:])
            r_sb = cpool.tile([128, tiles * F], F16)
            awh_sb = cpool.tile([128, F], F16)
            nc.vector.tensor_copy(awh_sb[:], awb_sb[:])
            if VARIANT == "compute_only":
                chc = cpool.tile([128, BATCH_COLS * 2 * F], F16)
                nc.vector.memset(chc[:], 0.25)

            if VARIANT == "empty":
                ot0 = spool.tile([128, F], F32, tag="ot")
                nc.vector.tensor_copy(ot0[:], awb_sb[:])
                for t in range(tiles):
                    nc.sync.dma_start(out=outp[t * 128:(t + 1) * 128, :], in_=ot0[:])
            # phase 1a: r_proj for local nodes, resident in SBUF
            for t in range(tiles if VARIANT != "empty" else 0):
                xt = xpool.tile([F + 1, 128], F16, tag="xl")
                nc.sync.dma_start(out=xt[:], in_=xlT[:, t * 128:(t + 1) * 128])
                ps = ppool.tile([128, F], F32, tag="psr")
                nc.tensor.matmul(ps[:], lhsT=xt[:], rhs=wrb_sb[:],
                                 start=True, stop=True)
                nc.scalar.copy(r_sb[:, t * F:(t + 1) * F], ps[:])

            # phase 1b: s_proj table in HBM (rows g*512+p*4+j for node
            # g*512+j*128+p -> 1KB contiguous per partition per group)
            for g in range(n_grp if VARIANT != "empty" else 0):
                xg = xpool.tile([F + 1, 512], F16, tag="xg")
                nc.sync.dma_start(out=xg[:], in_=xT[:, g * 512:(g + 1) * 512])
                ps = ppool.tile([128, 4 * F], F32, tag="pss")
                for j in range(4):
                    nc.tensor.matmul(
                        ps[:, j * F:(j + 1) * F],
                        lhsT=xg[:, j * 128:(j + 1) * 128],
                        rhs=wsb_sb[:], start=True, stop=True)
                sg = xpool.tile([128, 4 * F], F16, tag="sg")
                nc.scalar.copy(sg[:], ps[:])
                nc.sync.dma_start(
                    out=table[g * 256:(g + 1) * 256, :].rearrange(
                        "(p two) c -> p two c", p=128),
                    in_=sg[:].rearrange("p (two c) -> p two c", two=2))

            if VARIANT == "phase1_only":
                for t in range(tiles):
                    otp = spool.tile([128, F], F32, tag="ot")
                    nc.vector.tensor_copy(otp[:], r_sb[:, t * F:(t + 1) * F])
                    nc.sync.dma_start(out=outp[t * 128:(t + 1) * 128, :],
                                      in_=otp[:])

            n_main = tiles if VARIANT in ("full", "gather_only", "compute_only") else 0

            # ---- phase 2: batched gathers + 3-stage pipelined edge math ----
            state = {}

            def gather_batch(bi, rep):
                a, b = batches[bi]
                C = BC[bi]
                num_idx = 128 * C + 1
                ncol = 8 * C + 1
                ch = pch.tile([128, (C + 1) * 2 * F], F16, tag="chb",
                              name=f"chb{bi}r{rep}")
                nc.gpsimd.dma_gather(
                    ch[:].rearrange("p (k c) -> p k c", c=2 * F),
                    table[BIAS:, :],
                    idx_sb[:, O16[bi]:O16[bi] + ncol],
                    num_idx, num_idx, 2 * F,
                    single_packet=False, queue_num=bi % 4,
                )
                return ch

            def s1(t, ch, rep=0):
                Dt = D_t[t]
                off = OFF[t]
                KC = Dt * F
                ch3 = ch[:].rearrange("p (k c) -> p k c", c=2 * F)
                o = 0 if VARIANT == "compute_only" else ob_t[t]
                if VARIANT == "gather_only":
                    otg = spool.tile([128, F], F32, tag="ot")
                    nc.vector.tensor_copy(otg[:], ch[:, o * 2 * F:o * 2 * F + F])
                    nc.sync.dma_start(out=outp[t * 128:(t + 1) * 128, :],
                                      in_=otg[:])
                    return None
                se = pse.tile([128, KC], F16, tag="se", name=f"se{t}r{rep}")
                se3 = se[:].rearrange("p (k c) -> p k c", c=F)
                nc.vector.tensor_copy(se3, ch3[:, o:o + Dt, 0:F])
                b0_b = b0_sb[:, off:off + Dt][:, :, None].to_broadcast(
                    [128, Dt, F])
                nc.vector.copy_predicated(se3, b0_b, ch3[:, o:o + Dt, F:2 * F])
                re_b = r_sb[:, t * F:(t + 1) * F][:, None, :].to_broadcast(
                    [128, Dt, F])
                z = pz.tile([128, KC], F16, tag="z", name=f"z{t}r{rep}")
                nc.vector.tensor_tensor(
                    out=z[:].rearrange("p (k c) -> p k c", c=F),
                    in0=se3, in1=re_b, op=ALU.add)
                # mish(z) = z * tanh(ln(1 + e^z)); fp16 inf chain gives the
                # correct asymptote m = z for large z
                et = pet.tile([128, KC], F16, tag="et")
                nc.scalar.activation(et[:], z[:], AT.Exp)
                sp = pet.tile([128, KC], F16, tag="sp")
                nc.scalar.activation(sp[:], et[:], AT.Ln, bias=1.0)
                th = pth.tile([128, KC], F16, tag="th", name=f"th{t}r{rep}")
                nc.scalar.activation(th[:], sp[:], AT.Tanh)
                return (se, z, th)

            def s2(t, st, rep=0):
                Dt = D_t[t]
                KC = Dt * F
                se, z, th = st
                aw_b = awh_sb[:][:, None, :].to_broadcast([128, Dt, F])
                za = pza.tile([128, KC], F16, tag="za")
                nc.vector.tensor_tensor(
                    out=za[:].rearrange("p (k c) -> p k c", c=F),
                    in0=z[:].rearrange("p (k c) -> p k c", c=F),
                    in1=aw_b, op=ALU.mult)
                mw = pza.tile([128, KC], F16, tag="mw")
                nc.vector.tensor_tensor(out=mw[:], in0=za[:], in1=th[:],
                                        op=ALU.mult)
                logits = spool.tile([128, Dt * H], F32, tag="logits",
                                    name=f"lg{t}r{rep}")
                nc.vector.tensor_reduce(
                    out=logits[:],
                    in_=mw[:].rearrange("p (k h d) -> p k h d", h=H, d=HD),
                    axis=mybir.AxisListType.X, op=ALU.add)
                # max-shift for fp16-safe softmax numerators (pads gather real
                # table rows, so their logits are bounded like real ones)
                lmax = spool.tile([128, H], F32, tag="lmax")
                nc.vector.tensor_reduce(
                    out=lmax[:],
                    in_=logits[:].rearrange("p (k h) -> p h k", h=H),
                    axis=mybir.AxisListType.X, op=ALU.max)
                lsh = spool.tile([128, Dt * H], F32, tag="lsh", name=f"ls{t}r{rep}")
                lmax_b = lmax[:][:, None, :].to_broadcast([128, Dt, H])
                nc.vector.tensor_tensor(
                    out=lsh[:].rearrange("p (k h) -> p k h", h=H),
                    in0=logits[:].rearrange("p (k h) -> p k h", h=H),
                    in1=lmax_b, op=ALU.subtract)
                ex = spool.tile([128, Dt * H], F32, tag="ex", name=f"ex{t}r{rep}")
                nc.scalar.activation(ex[:], lsh[:], AT.Exp)
                return (se, ex)

            def s3(t, st):
                Dt = D_t[t]
                off = OFF[t]
                KC = Dt * F
                se, ex = st
                exm = spool.tile([128, Dt * H], F32, tag="exm")
                mask_b = mask_sb[:, off:off + Dt][:, :, None].to_broadcast(
                    [128, Dt, H])
                nc.vector.tensor_tensor(
                    out=exm[:].rearrange("p (k h) -> p k h", h=H),
                    in0=ex[:].rearrange("p (k h) -> p k h", h=H),
                    in1=mask_b, op=ALU.mult)
                den = spool.tile([128, H], F32, tag="den")
                nc.vector.tensor_reduce(
                    out=den[:],
                    in_=exm[:].rearrange("p (k h) -> p h k", h=H),
                    axis=mybir.AxisListType.X, op=ALU.add)
                # guard: zero-degree receivers must yield 0, not NaN
                deng = spool.tile([128, H], F32, tag="deng")
                nc.vector.tensor_scalar_add(deng[:], in0=den[:], scalar1=1e-30)
                rec = spool.tile([128, H], F32, tag="rec")
                nc.vector.reciprocal(rec[:], deng[:])
                exm16 = spool.tile([128, Dt * H], F16, tag="exm16")
                nc.scalar.copy(exm16[:], exm[:])
                wse = pwse.tile([128, KC], F16, tag="wse")
                exm_b = exm16[:].rearrange(
                    "p (k h) -> p k h", h=H)[:, :, :, None].to_broadcast(
                    [128, Dt, H, HD])
                nc.vector.tensor_tensor(
                    out=wse[:].rearrange("p (k h d) -> p k h d", h=H, d=HD),
                    in0=se[:].rearrange("p (k h d) -> p k h d", h=H, d=HD),
                    in1=exm_b, op=ALU.mult)
                num = spool.tile([128, F], F32, tag="num")
                nc.vector.tensor_reduce(
                    out=num[:],
                    in_=wse[:].rearrange("p (k c) -> p c k", c=F),
                    axis=mybir.AxisListType.X, op=ALU.add)
                ot = spool.tile([128, F], F32, tag="ot")
                rec_b = rec[:][:, :, None].to_broadcast([128, H, HD])
                nc.vector.tensor_tensor(
                    out=ot[:].rearrange("p (h d) -> p h d", h=H),
                    in0=num[:].rearrange("p (h d) -> p h d", h=H),
                    in1=rec_b, op=ALU.mult)
                nc.sync.dma_start(out=outp[t * 128:(t + 1) * 128, :], in_=ot[:])

            for rep in range(P2REPS if n_main else 0):
                st1 = {}
                st2 = {}
                cur_ch = {}
                for t in range(n_main + 2):
                    if t < n_main:
                        bi = bat_of[t]
                        if VARIANT == "compute_only":
                            ch = chc
                        elif bi in cur_ch:
                            ch = cur_ch[bi]
                        else:
                            ch = gather_batch(bi, rep)
                            cur_ch.clear()
                            cur_ch[bi] = ch
                        r = s1(t, ch, rep)
                        if r is not None:
                            st1[t] = r
                    if VARIANT == "gather_only":
                        continue
                    if t - 1 >= 0 and (t - 1) in st1:
                        st2[t - 1] = s2(t - 1, st1.pop(t - 1), rep)
                    if t - 2 >= 0 and (t - 2) in st2:
                        s3(t - 2, st2.pop(t - 2))

    return nc


def kernel(x, Ws, bs, Wr, br, aw, ab, senders, receivers):
    x = np.asarray(x, np.float32)
    senders = np.asarray(senders, np.int32)
    receivers = np.asarray(receivers, np.int32)
    ins, meta = _host_prep(x, np.asarray(Ws), np.asarray(bs), np.asarray(Wr),
                           np.asarray(br), np.asarray(aw), np.asarray(ab),
                           senders, receivers)
    nc = _build_program(meta)
    if not nc.is_finalized():
        nc.finalize()
    in_maps = []
    for c in range(NC_CORES):
        in_maps.append({
            "xT": ins["xT"],
            "xlT": ins["xlT"][c],
            "Wsb": ins["Wsb"],
            "Wrb": ins["Wrb"],
            "awb": ins["awb"],
            "idx16": ins["idx16"][c],
            "b0": ins["b0"][c],
            "mask": ins["mask"][c],
        })
    res = run_bass_kernel_spmd(nc, in_maps, core_ids=list(range(NC_CORES)))
    N = x.shape[0]
    order = meta["order"]
    out_full = np.zeros((N, F), dtype=np.float32)
    for c in range(NC_CORES):
        rows = order[c::NC_CORES]
        out_full[rows] = res.results[c]["out"][: len(rows)]
    return out_full


# revision 13
# speedup vs baseline: 1.3334x; 1.3334x over previous
"""GATv2Conv message-passing kernel for 8 Trainium2 NeuronCores.

Strategy (receiver-sharded, padded-grid, no collectives):
- Nodes are sorted by in-degree and dealt round-robin to the 8 cores, so each
  core owns ~12.5k receiver nodes with a balanced edge count, and consecutive
  128-node tiles have near-uniform degree (padding ratio ~1.02).
- Each core computes the full sender projection table s_proj = [x|1] @ [Ws;bs]
  on-device into an HBM scratch table (replicated work, fp16, viewed as 50176
  chunks of 2 rows / 256B), and its local receiver projection r_proj into SBUF.
- Per 128-node tile, sender rows are fetched with ONE InstDMAGatherAnt
  (dma_gather) on a rotating SWDGE queue (0-3): 128*D_t+1 int16 indices
  biased by -32768 (table base offset +32768 chunks) so the whole 50176-chunk
  table is reachable; the trailing index is a dummy with positive biased value
  so the ucode's trailing-negative trim never eats real slots. Each descriptor
  fetches a 256B chunk (2 rows); a uint8 parity plane + copy_predicated picks
  the right row. Then the GATv2 edge math (mish via Exp/Ln/Tanh LUTs, masked
  softmax without max-subtraction -- logits are O(5) here -- and the weighted
  aggregation) runs as dense DVE/ACT ops over the [128, D_t*64] grid.

Measured hardware facts that shaped this design (axon TRN2 microbenchmarks):
- indirect_dma_start costs ~9.4ns/descriptor (scalar Q7 SWDGE desc-gen, one
  index per partition per instruction) -> 1.6ms for 203k edge descriptors.
- dma_gather desc-gen is 16-lane vectorized on Q7 and queue_num q runs on Q7
  core pair (2q, 2q+1): 1 queue = 6.6ns/desc, 4 queues = 2.7ns/desc,
  independent of 256B vs 512B descriptor size (descriptor-path bound).
- copy_predicated requires a uint8 mask; Tanh LUT exists (fp16 4e-4), Mish
  and Softplus LUTs do not exist in this build.
- Manual software-pipelining of the per-tile stages and multi-tile batched
  gathers both measured SLOWER than this simple per-tile issue order (the
  tile scheduler's own pipelining + 6-deep gather buffers win).
"""

import numpy as np

import concourse.bass as bass
import concourse.bacc as bacc
import concourse.mybir as mybir
import concourse.tile as tile
from concourse.bass_utils import run_bass_kernel_spmd

F32 = mybir.dt.float32
F16 = mybir.dt.float16
I16 = mybir.dt.int16
U8 = mybir.dt.uint8

N_NODES = 100000
N_EDGES = 1600000
F = 64
H = 4
HD = 16
NC_CORES = 8
BIAS = 32768
DUMMY_CHUNK = 40000  # any chunk in [32768, 50176): positive biased value


def _host_prep(x, Ws, bs, Wr, br, aw, ab, senders, receivers):
    """Pure index/layout work: shard nodes+edges, build grid slot arrays."""
    N = x.shape[0]
    deg = np.bincount(receivers, minlength=N)
    order = np.argsort(deg, kind="stable").astype(np.int64)  # rank -> node
    inv_order = np.empty(N, dtype=np.int64)
    inv_order[order] = np.arange(N)

    rows_per_core = -(-N // NC_CORES)          # 12500
    tiles = -(-rows_per_core // 128)           # 98
    rows_pad = tiles * 128                     # 12544

    # per-tile max degree over the 1024-rank window (common across cores)
    d_pad = np.zeros(tiles * 1024, dtype=np.int64)
    d_pad[: N] = deg[order]
    D_t = d_pad.reshape(tiles, 1024).max(axis=1)
    D_t = np.maximum(D_t, 1)
    OFF = np.concatenate([[0], np.cumsum(D_t)]).astype(np.int64)
    S = int(OFF[-1])
    # int16 idx plane offsets: per tile 8*D_t + 1 columns
    O16 = np.concatenate([[0], np.cumsum(8 * D_t + 1)]).astype(np.int64)
    SI16 = int(O16[-1])

    # edge -> (core, row, k)
    erank = inv_order[receivers]
    e_sort = np.argsort(erank, kind="stable")
    er_sorted = erank[e_sort]
    s_sorted = senders[e_sort]
    grp_start = np.searchsorted(er_sorted, np.arange(N))
    k_all = np.arange(len(er_sorted)) - grp_start[er_sorted]

    core_e = er_sorted % NC_CORES
    row_e = er_sorted // NC_CORES
    t_e = row_e // 128
    p_e = row_e % 128
    col_e = OFF[t_e] + k_all

    # sender node n = g*512 + j*128 + p lands at table row g*512 + p*4 + j
    # (lets phase-1b write 1KB-contiguous runs per partition); chunk = row//2
    g_n = s_sorted // 512
    rem = s_sorted % 512
    j_n = rem // 128
    p_n = rem % 128
    tau = g_n * 512 + p_n * 4 + j_n
    chunk_e = tau // 2
    par_e = (tau % 2).astype(np.uint8)

    mask_arr = np.zeros((NC_CORES, 128, S), dtype=np.float32)
    b0_arr = np.zeros((NC_CORES, 128, S), dtype=np.uint8)
    mask_arr[core_e, p_e, col_e] = 1.0
    b0_arr[core_e, p_e, col_e] = par_e

    # int16 gather index planes: per tile the flat order is i = c*128 + p,
    # wrapped 16-wide ((partition i%16, col i//16), replicated x8 groups)
    chunk_grid = np.full((NC_CORES, 128, S), DUMMY_CHUNK, dtype=np.int64)
    chunk_grid[core_e, p_e, col_e] = chunk_e
    idx16_arr = np.zeros((NC_CORES, 128, SI16), dtype=np.int16)
    for t in range(tiles):
        Dt = int(D_t[t])
        ncol = 8 * Dt + 1
        flat = np.empty((NC_CORES, ncol * 16), dtype=np.int16)
        flat[:, :] = np.int16(DUMMY_CHUNK - BIAS)
        fl = (chunk_grid[:, :, OFF[t]:OFF[t] + Dt] - BIAS).astype(np.int16)
        flat[:, : 128 * Dt] = fl.transpose(0, 2, 1).reshape(NC_CORES, -1)
        wrap = flat.reshape(NC_CORES, ncol, 16).transpose(0, 2, 1)
        idx16_arr[:, :, O16[t]:O16[t] + ncol] = np.tile(wrap, (1, 8, 1))

    # x^T padded + ones row, shared across cores
    n_grp = -(-N // 512)
    n_tab = n_grp * 512
    n_chunks = n_tab // 2
    xT_aug = np.zeros((F + 1, n_tab), dtype=np.float16)
    xT_aug[:F, :N] = x.T
    xT_aug[F, :] = 1.0

    # per-core local x^T (+ones)
    xlT = np.zeros((NC_CORES, F + 1, rows_pad), dtype=np.float16)
    for c in range(NC_CORES):
        rows = order[c::NC_CORES]          # ranks c, c+8, ... ascending rank
        xlT[c, :F, : len(rows)] = x[rows].T
        xlT[c, F, :] = 1.0

    Wsb = np.concatenate([Ws.reshape(F, F), bs.reshape(1, F)], axis=0).astype(np.float16)
    Wrb = np.concatenate([Wr.reshape(F, F), br.reshape(1, F)], axis=0).astype(np.float16)
    aw_rep = np.tile(np.asarray(aw, np.float32).reshape(1, HD), (1, H)).reshape(1, F)
    awb = np.tile(aw_rep, (128, 1)).astype(np.float32)

    meta = dict(
        D_t=D_t.astype(int).tolist(),
        OFF=OFF.astype(int).tolist(),
        O16=O16.astype(int).tolist(),
        S=S,
        SI16=SI16,
        tiles=tiles,
        rows_pad=rows_pad,
        n_tab=n_tab,
        n_chunks=n_chunks,
        n_grp=n_grp,
        order=order,
        ab=float(np.asarray(ab).reshape(-1)[0]),
    )
    ins = dict(xT=xT_aug, xlT=xlT, Wsb=Wsb, Wrb=Wrb, awb=awb,
               idx16=idx16_arr, b0=b0_arr, mask=mask_arr)
    return ins, meta


VARIANT = "full"  # full | gather_only | compute_only | phase1_only | empty
P2REPS = 1        # repeat phase-2 (timing experiments only)


def _build_program(meta):
    D_t, OFF, O16, S, SI16 = (meta["D_t"], meta["OFF"], meta["O16"],
                              meta["S"], meta["SI16"])
    tiles, rows_pad, n_tab, n_chunks, n_grp = (
        meta["tiles"], meta["rows_pad"], meta["n_tab"], meta["n_chunks"],
        meta["n_grp"])

    nc = bacc.Bacc(num_swdge_queues=4)
    xT = nc.declare_dram_parameter("xT", [F + 1, n_tab], F16, isOutput=False)
    xlT = nc.declare_dram_parameter("xlT", [F + 1, rows_pad], F16, isOutput=False)
    Wsb = nc.declare_dram_parameter("Wsb", [F + 1, F], F16, isOutput=False)
    Wrb = nc.declare_dram_parameter("Wrb", [F + 1, F], F16, isOutput=False)
    awb = nc.declare_dram_parameter("awb", [128, F], F32, isOutput=False)
    idx16p = nc.declare_dram_parameter("idx16", [128, SI16], I16, isOutput=False)
    b0p = nc.declare_dram_parameter("b0", [128, S], U8, isOutput=False)
    maskp = nc.declare_dram_parameter("mask", [128, S], F32, isOutput=False)
    outp = nc.declare_dram_parameter("out", [rows_pad, F], F32, isOutput=True)

    AT = mybir.ActivationFunctionType
    ALU = mybir.AluOpType

    with tile.TileContext(nc) as tc:
        with (
            tc.tile_pool(name="dram", bufs=1, space="DRAM") as dpool,
            tc.tile_pool(name="consts", bufs=1) as cpool,
            tc.tile_pool(name="xload", bufs=3) as xpool,
            tc.tile_pool(name="pch", bufs=6) as pch,
            tc.tile_pool(name="pz", bufs=3) as pz,
            tc.tile_pool(name="pa", bufs=3) as pa,
            tc.tile_pool(name="pb", bufs=3) as pb,
            tc.tile_pool(name="small", bufs=4) as spool,
            tc.tile_pool(name="psum", bufs=2, space="PSUM") as ppool,
        ):
            # s_proj table viewed as 2-row (256B) chunks
            table = dpool.tile([n_chunks, 2 * F], F16)

            wsb_sb = cpool.tile([F + 1, F], F16)
            nc.sync.dma_start(out=wsb_sb[:], in_=Wsb[:])
            wrb_sb = cpool.tile([F + 1, F], F16)
            nc.sync.dma_start(out=wrb_sb[:], in_=Wrb[:])
            awb_sb = cpool.tile([128, F], F32)
            nc.sync.dma_start(out=awb_sb[:], in_=awb[:])
            idx_sb = cpool.tile([128, SI16], I16)
            nc.sync.dma_start(out=idx_sb[:], in_=idx16p[:])
            b0_sb = cpool.tile([128, S], U8)
            nc.sync.dma_start(out=b0_sb[:], in_=b0p[:])
            mask_sb = cpool.tile([128, S], F32)
            nc.sync.dma_start(out=mask_sb[:], in_=maskp[:])
            r_sb = cpool.tile([128, tiles * F], F16)
            awh_sb = cpool.tile([128, F], F16)
            nc.vector.tensor_copy(awh_sb[:], awb_sb[:])
            if VARIANT == "compute_only":
                chc = cpool.tile([128, (max(D_t) + 1) * 2 * F], F16)
                nc.vector.memset(chc[:], 0.25)

            if VARIANT == "empty":
                ot0 = spool.tile([128, F], F32, tag="ot")
                nc.vector.tensor_copy(ot0[:], awb_sb[:])
                for t in range(tiles):
                    nc.sync.dma_start(out=outp[t * 128:(t + 1) * 128, :], in_=ot0[:])
            # phase 1a: r_proj for local nodes, resident in SBUF
            for t in range(tiles if VARIANT != "empty" else 0):
                xt = xpool.tile([F + 1, 128], F16, tag="xl")
                nc.sync.dma_start(out=xt[:], in_=xlT[:, t * 128:(t + 1) * 128])
                ps = ppool.tile([128, F], F32, tag="psr")
                nc.tensor.matmul(ps[:], lhsT=xt[:], rhs=wrb_sb[:],
                                 start=True, stop=True)
                nc.scalar.copy(r_sb[:, t * F:(t + 1) * F], ps[:])

            # phase 1b: s_proj table in HBM (rows g*512+p*4+j for node
            # g*512+j*128+p -> 1KB contiguous per partition per group)
            for g in range(n_grp if VARIANT != "empty" else 0):
                xg = xpool.tile([F + 1, 512], F16, tag="xg")
                nc.sync.dma_start(out=xg[:], in_=xT[:, g * 512:(g + 1) * 512])
                ps = ppool.tile([128, 4 * F], F32, tag="pss")
                for j in range(4):
                    nc.tensor.matmul(
                        ps[:, j * F:(j + 1) * F],
                        lhsT=xg[:, j * 128:(j + 1) * 128],
                        rhs=wsb_sb[:], start=True, stop=True)
                sg = xpool.tile([128, 4 * F], F16, tag="sg")
                nc.vector.tensor_copy(sg[:], ps[:])
                nc.sync.dma_start(
                    out=table[g * 256:(g + 1) * 256, :].rearrange(
                        "(p two) c -> p two c", p=128),
                    in_=sg[:].rearrange("p (two c) -> p two c", two=2))

            if VARIANT == "phase1_only":
                for t in range(tiles):
                    otp = spool.tile([128, F], F32, tag="ot")
                    nc.vector.tensor_copy(otp[:], r_sb[:, t * F:(t + 1) * F])
                    nc.sync.dma_start(out=outp[t * 128:(t + 1) * 128, :],
                                      in_=otp[:])

            n_main = tiles if VARIANT in ("full", "gather_only", "compute_only") else 0

            def compute_tile(t, ch, rep):
                Dt = D_t[t]
                off = OFF[t]
                KC = Dt * F
                ch3 = ch[:].rearrange("p (k c) -> p k c", c=2 * F)
                if VARIANT == "gather_only":
                    otg = spool.tile([128, F], F32, tag="ot")
                    nc.vector.tensor_copy(otg[:], ch[:, :F])
                    nc.sync.dma_start(out=outp[t * 128:(t + 1) * 128, :],
                                      in_=otg[:])
                    return
                # parity select: se = parity ? chunk[64:128] : chunk[0:64]
                se = pz.tile([128, KC], F16, tag="se", name=f"se{t}r{rep}")
                se3 = se[:].rearrange("p (k c) -> p k c", c=F)
                nc.vector.tensor_copy(se3, ch3[:, :Dt, 0:F])
                b0_b = b0_sb[:, off:off + Dt][:, :, None].to_broadcast(
                    [128, Dt, F])
                nc.vector.copy_predicated(se3, b0_b, ch3[:, :Dt, F:2 * F])
                re_b = r_sb[:, t * F:(t + 1) * F][:, None, :].to_broadcast(
                    [128, Dt, F])
                z = pa.tile([128, KC], F16, tag="A")
                nc.vector.tensor_tensor(
                    out=z[:].rearrange("p (k c) -> p k c", c=F),
                    in0=se3, in1=re_b, op=ALU.add)
                # mish(z) = z * tanh(ln(1 + e^z)); fp16 inf chain gives the
                # correct asymptote m = z for large z
                et = pb.tile([128, KC], F16, tag="B")
                nc.scalar.activation(et[:], z[:], AT.Exp)
                sp = pa.tile([128, KC], F16, tag="A2")
                nc.scalar.activation(sp[:], et[:], AT.Ln, bias=1.0)
                th = pb.tile([128, KC], F16, tag="B2")
                nc.scalar.activation(th[:], sp[:], AT.Tanh)
                aw_b = awh_sb[:][:, None, :].to_broadcast([128, Dt, F])
                za = pa.tile([128, KC], F16, tag="A")
                nc.vector.tensor_tensor(
                    out=za[:].rearrange("p (k c) -> p k c", c=F),
                    in0=z[:].rearrange("p (k c) -> p k c", c=F),
                    in1=aw_b, op=ALU.mult)
                mw = pb.tile([128, KC], F16, tag="B")
                nc.vector.tensor_tensor(out=mw[:], in0=za[:], in1=th[:],
                                        op=ALU.mult)
                logits = spool.tile([128, Dt * H], F32, tag="logits")
                nc.vector.tensor_reduce(
                    out=logits[:],
                    in_=mw[:].rearrange("p (k h d) -> p k h d", h=H, d=HD),
                    axis=mybir.AxisListType.X, op=ALU.add)
                # ab cancels in the softmax (constant shift) -- skip it
                ex = spool.tile([128, Dt * H], F32, tag="ex")
                nc.scalar.activation(ex[:], logits[:], AT.Exp)
                exm = spool.tile([128, Dt * H], F32, tag="exm")
                mask_b = mask_sb[:, off:off + Dt][:, :, None].to_broadcast(
                    [128, Dt, H])
                nc.vector.tensor_tensor(
                    out=exm[:].rearrange("p (k h) -> p k h", h=H),
                    in0=ex[:].rearrange("p (k h) -> p k h", h=H),
                    in1=mask_b, op=ALU.mult)
                den = spool.tile([128, H], F32, tag="den")
                nc.vector.tensor_reduce(
                    out=den[:],
                    in_=exm[:].rearrange("p (k h) -> p h k", h=H),
                    axis=mybir.AxisListType.X, op=ALU.add)
                # guard: zero-degree receivers must yield 0, not NaN
                deng = spool.tile([128, H], F32, tag="deng")
                nc.vector.tensor_scalar_add(deng[:], in0=den[:], scalar1=1e-30)
                rec = spool.tile([128, H], F32, tag="rec")
                nc.vector.reciprocal(rec[:], deng[:])
                wse = pb.tile([128, KC], F32, tag="W")
                exm_b = exm[:].rearrange(
                    "p (k h) -> p k h", h=H)[:, :, :, None].to_broadcast(
                    [128, Dt, H, HD])
                nc.vector.tensor_tensor(
                    out=wse[:].rearrange("p (k h d) -> p k h d", h=H, d=HD),
                    in0=se[:].rearrange("p (k h d) -> p k h d", h=H, d=HD),
                    in1=exm_b, op=ALU.mult)
                num = spool.tile([128, F], F32, tag="num")
                nc.vector.tensor_reduce(
                    out=num[:],
                    in_=wse[:].rearrange("p (k c) -> p c k", c=F),
                    axis=mybir.AxisListType.X, op=ALU.add)
                ot = spool.tile([128, F], F32, tag="ot")
                rec_b = rec[:][:, :, None].to_broadcast([128, H, HD])
                nc.vector.tensor_tensor(
                    out=ot[:].rearrange("p (h d) -> p h d", h=H),
                    in0=num[:].rearrange("p (h d) -> p h d", h=H),
                    in1=rec_b, op=ALU.mult)
                nc.sync.dma_start(out=outp[t * 128:(t + 1) * 128, :], in_=ot[:])

            for rep in range(P2REPS if n_main else 0):
                for t in range(n_main):
                    Dt = D_t[t]
                    num_idx = 128 * Dt + 1
                    ncol = 8 * Dt + 1
                    if VARIANT == "compute_only":
                        ch = chc
                    else:
                        ch = pch.tile([128, (Dt + 1) * 2 * F], F16, tag="ch",
                                      name=f"ch{t}r{rep}")
                        nc.gpsimd.dma_gather(
                            ch[:].rearrange("p (k c) -> p k c", c=2 * F),
                            table[BIAS:, :],
                            idx_sb[:, O16[t]:O16[t] + ncol],
                            num_idx, num_idx, 2 * F,
                            single_packet=False, queue_num=t % 4,
                        )
                    compute_tile(t, ch, rep)

    return nc


def kernel(x, Ws, bs, Wr, br, aw, ab, senders, receivers):
    x = np.asarray(x, np.float32)
    senders = np.asarray(senders, np.int32)
    receivers = np.asarray(receivers, np.int32)
    ins, meta = _host_prep(x, np.asarray(Ws), np.asarray(bs), np.asarray(Wr),
                           np.asarray(br), np.asarray(aw), np.asarray(ab),
                           senders, receivers)
    nc = _build_program(meta)
    if not nc.is_finalized():
        nc.finalize()
    in_maps = []
    for c in range(NC_CORES):
        in_maps.append({
            "xT": ins["xT"],
            "xlT": ins["xlT"][c],
            "Wsb": ins["Wsb"],
            "Wrb": ins["Wrb"],
            "awb": ins["awb"],
            "idx16": ins["idx16"][c],
            "b0": ins["b0"][c],
            "mask": ins["mask"][c],
        })
    res = run_bass_kernel_spmd(nc, in_maps, core_ids=list(range(NC_CORES)))
    N = x.shape[0]
    order = meta["order"]
    out_full = np.zeros((N, F), dtype=np.float32)
    for c in range(NC_CORES):
        rows = order[c::NC_CORES]
        out_full[rows] = res.results[c]["out"][: len(rows)]
    return out_full


# revision 17
# speedup vs baseline: 1.8163x; 1.3621x over previous
"""GATv2Conv message-passing kernel for 8 Trainium2 NeuronCores.

Strategy (receiver-sharded, padded-grid, no collectives):
- Nodes are sorted by in-degree and dealt round-robin to the 8 cores, so each
  core owns ~12.5k receiver nodes with a balanced edge count, and consecutive
  128-node tiles have near-uniform degree (padding ratio ~1.02).
- Each core computes the full sender projection table s_proj = [x|1] @ [Ws;bs]
  on-device into an HBM scratch table (replicated work, fp16, viewed as 50176
  chunks of 2 rows / 256B), and its local receiver projection r_proj into SBUF.
- Per 128-node tile, sender rows are fetched with ONE InstDMAGatherAnt
  (dma_gather) on a rotating SWDGE queue (0-3): 128*D_t+1 int16 indices
  biased by -32768 (table base offset +32768 chunks) so the whole 50176-chunk
  table is reachable; the trailing index is a dummy with positive biased value
  so the ucode's trailing-negative trim never eats real slots. Each descriptor
  fetches a 256B chunk (2 rows); a uint8 parity plane + copy_predicated picks
  the right row. Then the GATv2 edge math (mish via Exp/Ln/Tanh LUTs, masked
  softmax without max-subtraction -- logits are O(5) here -- and the weighted
  aggregation) runs as dense DVE/ACT ops over the [128, D_t*64] grid.

Measured hardware facts that shaped this design (axon TRN2 microbenchmarks):
- indirect_dma_start costs ~9.4ns/descriptor (scalar Q7 SWDGE desc-gen, one
  index per partition per instruction) -> 1.6ms for 203k edge descriptors.
- dma_gather desc-gen is 16-lane vectorized on Q7 and queue_num q runs on Q7
  core pair (2q, 2q+1): 1 queue = 6.6ns/desc, 4 queues = 2.7ns/desc,
  independent of 256B vs 512B descriptor size (descriptor-path bound).
- copy_predicated requires a uint8 mask; Tanh LUT exists (fp16 4e-4), Mish
  and Softplus LUTs do not exist in this build.
- Manual software-pipelining of the per-tile stages and multi-tile batched
  gathers both measured SLOWER than this simple per-tile issue order (the
  tile scheduler's own pipelining + 6-deep gather buffers win).
"""

import numpy as np

import concourse.bass as bass
import concourse.bacc as bacc
import concourse.mybir as mybir
import concourse.tile as tile
from concourse.bass_utils import run_bass_kernel_spmd

F32 = mybir.dt.float32
F16 = mybir.dt.float16
I16 = mybir.dt.int16
U8 = mybir.dt.uint8

N_NODES = 100000
N_EDGES = 1600000
F = 64
H = 4
HD = 16
NC_CORES = 8
BIAS = 32768
DUMMY_CHUNK = 40000  # any chunk in [32768, 50176): positive biased value


def _host_prep(x, Ws, bs, Wr, br, aw, ab, senders, receivers):
    """Pure index/layout work: shard nodes+edges, build grid slot arrays."""
    N = x.shape[0]
    deg = np.bincount(receivers, minlength=N)
    order = np.argsort(deg, kind="stable").astype(np.int64)  # rank -> node
    inv_order = np.empty(N, dtype=np.int64)
    inv_order[order] = np.arange(N)

    rows_per_core = -(-N // NC_CORES)          # 12500
    tiles = -(-rows_per_core // 128)           # 98
    rows_pad = tiles * 128                     # 12544

    # per-tile max degree over the 1024-rank window (common across cores)
    d_pad = np.zeros(tiles * 1024, dtype=np.int64)
    d_pad[: N] = deg[order]
    D_t = d_pad.reshape(tiles, 1024).max(axis=1)
    D_t = np.maximum(D_t, 1)
    OFF = np.concatenate([[0], np.cumsum(D_t)]).astype(np.int64)
    S = int(OFF[-1])
    # int16 idx plane offsets: per tile 8*D_t + 1 columns
    O16 = np.concatenate([[0], np.cumsum(8 * D_t + 1)]).astype(np.int64)
    SI16 = int(O16[-1])

    # edge -> (core, row, k)
    erank = inv_order[receivers]
    e_sort = np.argsort(erank, kind="stable")
    er_sorted = erank[e_sort]
    s_sorted = senders[e_sort]
    grp_start = np.searchsorted(er_sorted, np.arange(N))
    k_all = np.arange(len(er_sorted)) - grp_start[er_sorted]

    core_e = er_sorted % NC_CORES
    row_e = er_sorted // NC_CORES
    t_e = row_e // 128
    p_e = row_e % 128
    col_e = OFF[t_e] + k_all

    # sender node n = g*512 + j*128 + p lands at table row g*512 + p*4 + j
    # (lets phase-1b write 1KB-contiguous runs per partition); chunk = row//2
    g_n = s_sorted // 512
    rem = s_sorted % 512
    j_n = rem // 128
    p_n = rem % 128
    tau = g_n * 512 + p_n * 4 + j_n
    chunk_e = tau // 2
    par_e = (tau % 2).astype(np.uint8)

    mask_arr = np.zeros((NC_CORES, 128, S), dtype=np.float32)
    b0_arr = np.zeros((NC_CORES, 128, S), dtype=np.uint8)
    mask_arr[core_e, p_e, col_e] = 1.0
    b0_arr[core_e, p_e, col_e] = par_e

    # int16 gather index planes: per tile the flat order is i = c*128 + p,
    # wrapped 16-wide ((partition i%16, col i//16), replicated x8 groups)
    chunk_grid = np.full((NC_CORES, 128, S), DUMMY_CHUNK, dtype=np.int64)
    chunk_grid[core_e, p_e, col_e] = chunk_e
    idx16_arr = np.zeros((NC_CORES, 128, SI16), dtype=np.int16)
    for t in range(tiles):
        Dt = int(D_t[t])
        ncol = 8 * Dt + 1
        flat = np.empty((NC_CORES, ncol * 16), dtype=np.int16)
        flat[:, :] = np.int16(DUMMY_CHUNK - BIAS)
        fl = (chunk_grid[:, :, OFF[t]:OFF[t] + Dt] - BIAS).astype(np.int16)
        flat[:, : 128 * Dt] = fl.transpose(0, 2, 1).reshape(NC_CORES, -1)
        wrap = flat.reshape(NC_CORES, ncol, 16).transpose(0, 2, 1)
        idx16_arr[:, :, O16[t]:O16[t] + ncol] = np.tile(wrap, (1, 8, 1))

    # x^T padded + ones row, shared across cores
    n_grp = -(-N // 512)
    n_tab = n_grp * 512
    n_chunks = n_tab // 2
    xT_aug = np.zeros((F + 1, n_tab), dtype=np.float16)
    xT_aug[:F, :N] = x.T
    xT_aug[F, :] = 1.0

    # per-core local x^T (+ones)
    xlT = np.zeros((NC_CORES, F + 1, rows_pad), dtype=np.float16)
    for c in range(NC_CORES):
        rows = order[c::NC_CORES]          # ranks c, c+8, ... ascending rank
        xlT[c, :F, : len(rows)] = x[rows].T
        xlT[c, F, :] = 1.0

    Wsb = np.concatenate([Ws.reshape(F, F), bs.reshape(1, F)], axis=0).astype(np.float16)
    Wrb = np.concatenate([Wr.reshape(F, F), br.reshape(1, F)], axis=0).astype(np.float16)
    aw_rep = np.tile(np.asarray(aw, np.float32).reshape(1, HD), (1, H)).reshape(1, F)
    awb = np.tile(aw_rep, (128, 1)).astype(np.float32)

    meta = dict(
        D_t=D_t.astype(int).tolist(),
        OFF=OFF.astype(int).tolist(),
        O16=O16.astype(int).tolist(),
        S=S,
        SI16=SI16,
        tiles=tiles,
        rows_pad=rows_pad,
        n_tab=n_tab,
        n_chunks=n_chunks,
        n_grp=n_grp,
        order=order,
        ab=float(np.asarray(ab).reshape(-1)[0]),
    )
    ins = dict(xT=xT_aug, xlT=xlT, Wsb=Wsb, Wrb=Wrb, awb=awb,
               idx16=idx16_arr, b0=b0_arr, mask=mask_arr)
    return ins, meta


VARIANT = "full"  # full | gather_only | compute_only | phase1_only | empty
P2REPS = 1        # repeat phase-2 (timing experiments only)


def _build_program(meta):
    D_t, OFF, O16, S, SI16 = (meta["D_t"], meta["OFF"], meta["O16"],
                              meta["S"], meta["SI16"])
    tiles, rows_pad, n_tab, n_chunks, n_grp = (
        meta["tiles"], meta["rows_pad"], meta["n_tab"], meta["n_chunks"],
        meta["n_grp"])

    nc = bacc.Bacc(num_swdge_queues=4)
    xT = nc.declare_dram_parameter("xT", [F + 1, n_tab], F16, isOutput=False)
    xlT = nc.declare_dram_parameter("xlT", [F + 1, rows_pad], F16, isOutput=False)
    Wsb = nc.declare_dram_parameter("Wsb", [F + 1, F], F16, isOutput=False)
    Wrb = nc.declare_dram_parameter("Wrb", [F + 1, F], F16, isOutput=False)
    awb = nc.declare_dram_parameter("awb", [128, F], F32, isOutput=False)
    idx16p = nc.declare_dram_parameter("idx16", [128, SI16], I16, isOutput=False)
    b0p = nc.declare_dram_parameter("b0", [128, S], U8, isOutput=False)
    maskp = nc.declare_dram_parameter("mask", [128, S], F32, isOutput=False)
    outp = nc.declare_dram_parameter("out", [rows_pad, F], F32, isOutput=True)

    AT = mybir.ActivationFunctionType
    ALU = mybir.AluOpType

    with tile.TileContext(nc) as tc:
        with (
            tc.tile_pool(name="dram", bufs=1, space="DRAM") as dpool,
            tc.tile_pool(name="consts", bufs=1) as cpool,
            tc.tile_pool(name="xload", bufs=3) as xpool,
            tc.tile_pool(name="pch", bufs=6) as pch,
            tc.tile_pool(name="pz", bufs=3) as pz,
            tc.tile_pool(name="pa", bufs=3) as pa,
            tc.tile_pool(name="pb", bufs=3) as pb,
            tc.tile_pool(name="small", bufs=4) as spool,
            tc.tile_pool(name="psum", bufs=2, space="PSUM") as ppool,
        ):
            # s_proj table viewed as 2-row (256B) chunks
            table = dpool.tile([n_chunks, 2 * F], F16)

            wsb_sb = cpool.tile([F + 1, F], F16)
            nc.sync.dma_start(out=wsb_sb[:], in_=Wsb[:])
            wrb_sb = cpool.tile([F + 1, F], F16)
            nc.sync.dma_start(out=wrb_sb[:], in_=Wrb[:])
            awb_sb = cpool.tile([128, F], F32)
            nc.sync.dma_start(out=awb_sb[:], in_=awb[:])
            idx_sb = cpool.tile([128, SI16], I16)
            nc.sync.dma_start(out=idx_sb[:], in_=idx16p[:])
            b0_sb = cpool.tile([128, S], U8)
            nc.sync.dma_start(out=b0_sb[:], in_=b0p[:])
            mask_sb = cpool.tile([128, S], F32)
            nc.sync.dma_start(out=mask_sb[:], in_=maskp[:])
            r_sb = cpool.tile([128, tiles * F], F16)
            awh_sb = cpool.tile([128, F], F16)
            nc.vector.tensor_copy(awh_sb[:], awb_sb[:])
            if VARIANT == "compute_only":
                chc = cpool.tile([128, (max(D_t) + 1) * 2 * F], F16)
                nc.vector.memset(chc[:], 0.25)

            if VARIANT == "empty":
                ot0 = spool.tile([128, F], F32, tag="ot")
                nc.vector.tensor_copy(ot0[:], awb_sb[:])
                for t in range(tiles):
                    nc.sync.dma_start(out=outp[t * 128:(t + 1) * 128, :], in_=ot0[:])
            # phase 1a: r_proj for local nodes, resident in SBUF
            for t in range(tiles if VARIANT != "empty" else 0):
                xt = xpool.tile([F + 1, 128], F16, tag="xl")
                nc.sync.dma_start(out=xt[:], in_=xlT[:, t * 128:(t + 1) * 128])
                ps = ppool.tile([128, F], F32, tag="psr")
                nc.tensor.matmul(ps[:], lhsT=xt[:], rhs=wrb_sb[:],
                                 start=True, stop=True)
                nc.scalar.copy(r_sb[:, t * F:(t + 1) * F], ps[:])

            # phase 1b: s_proj table in HBM (rows g*512+p*4+j for node
            # g*512+j*128+p -> 1KB contiguous per partition per group)
            for g in range(n_grp if VARIANT != "empty" else 0):
                xg = xpool.tile([F + 1, 512], F16, tag="xg")
                nc.sync.dma_start(out=xg[:], in_=xT[:, g * 512:(g + 1) * 512])
                ps = ppool.tile([128, 4 * F], F32, tag="pss")
                for j in range(4):
                    nc.tensor.matmul(
                        ps[:, j * F:(j + 1) * F],
                        lhsT=xg[:, j * 128:(j + 1) * 128],
                        rhs=wsb_sb[:], start=True, stop=True)
                sg = xpool.tile([128, 4 * F], F16, tag="sg")
                nc.vector.tensor_copy(sg[:], ps[:])
                nc.sync.dma_start(
                    out=table[g * 256:(g + 1) * 256, :].rearrange(
                        "(p two) c -> p two c", p=128),
                    in_=sg[:].rearrange("p (two c) -> p two c", two=2))

            if VARIANT == "phase1_only":
                for t in range(tiles):
                    otp = spool.tile([128, F], F32, tag="ot")
                    nc.vector.tensor_copy(otp[:], r_sb[:, t * F:(t + 1) * F])
                    nc.sync.dma_start(out=outp[t * 128:(t + 1) * 128, :],
                                      in_=otp[:])

            n_main = tiles if VARIANT in ("full", "gather_only", "compute_only") else 0

            def compute_tile(t, ch, rep):
                Dt = D_t[t]
                off = OFF[t]
                KC = Dt * F
                ch3 = ch[:].rearrange("p (k c) -> p k c", c=2 * F)
                if VARIANT == "gather_only":
                    otg = spool.tile([128, F], F32, tag="ot")
                    nc.vector.tensor_copy(otg[:], ch[:, :F])
                    nc.sync.dma_start(out=outp[t * 128:(t + 1) * 128, :],
                                      in_=otg[:])
                    return
                # parity select: se = parity ? chunk[64:128] : chunk[0:64]
                se = pz.tile([128, KC], F16, tag="se", name=f"se{t}r{rep}")
                se3 = se[:].rearrange("p (k c) -> p k c", c=F)
                nc.vector.tensor_copy(se3, ch3[:, :Dt, 0:F])
                b0_b = b0_sb[:, off:off + Dt][:, :, None].to_broadcast(
                    [128, Dt, F])
                nc.vector.copy_predicated(se3, b0_b, ch3[:, :Dt, F:2 * F])
                re_b = r_sb[:, t * F:(t + 1) * F][:, None, :].to_broadcast(
                    [128, Dt, F])
                z = pa.tile([128, KC], F16, tag="A")
                nc.vector.tensor_tensor(
                    out=z[:].rearrange("p (k c) -> p k c", c=F),
                    in0=se3, in1=re_b, op=ALU.add)
                # mish(z) = z * tanh(ln(1 + e^z)); fp16 inf chain gives the
                # correct asymptote m = z for large z
                et = pb.tile([128, KC], F16, tag="B")
                nc.scalar.activation(et[:], z[:], AT.Exp)
                sp = pa.tile([128, KC], F16, tag="A2")
                nc.scalar.activation(sp[:], et[:], AT.Ln, bias=1.0)
                th = pb.tile([128, KC], F16, tag="B2")
                nc.scalar.activation(th[:], sp[:], AT.Tanh)
                aw_b = awh_sb[:][:, None, :].to_broadcast([128, Dt, F])
                za = pa.tile([128, KC], F16, tag="A")
                nc.vector.tensor_tensor(
                    out=za[:].rearrange("p (k c) -> p k c", c=F),
                    in0=z[:].rearrange("p (k c) -> p k c", c=F),
                    in1=aw_b, op=ALU.mult)
                mw = pb.tile([128, KC], F16, tag="B")
                nc.vector.tensor_tensor(out=mw[:], in0=za[:], in1=th[:],
                                        op=ALU.mult)
                logits = spool.tile([128, Dt * H], F32, tag="logits")
                nc.vector.tensor_reduce(
                    out=logits[:],
                    in_=mw[:].rearrange("p (k h d) -> p k h d", h=H, d=HD),
                    axis=mybir.AxisListType.X, op=ALU.add)
                # ab cancels in the softmax (constant shift) -- skip it
                ex = spool.tile([128, Dt * H], F32, tag="ex")
                nc.scalar.activation(ex[:], logits[:], AT.Exp)
                exm = spool.tile([128, Dt * H], F32, tag="exm")
                mask_b = mask_sb[:, off:off + Dt][:, :, None].to_broadcast(
                    [128, Dt, H])
                nc.vector.tensor_tensor(
                    out=exm[:].rearrange("p (k h) -> p k h", h=H),
                    in0=ex[:].rearrange("p (k h) -> p k h", h=H),
                    in1=mask_b, op=ALU.mult)
                den = spool.tile([128, H], F32, tag="den")
                nc.vector.tensor_reduce(
                    out=den[:],
                    in_=exm[:].rearrange("p (k h) -> p h k", h=H),
                    axis=mybir.AxisListType.X, op=ALU.add)
                # guard: zero-degree receivers must yield 0, not NaN
                deng = spool.tile([128, H], F32, tag="deng")
                nc.vector.tensor_scalar_add(deng[:], in0=den[:], scalar1=1e-30)
                rec = spool.tile([128, H], F32, tag="rec")
                nc.vector.reciprocal(rec[:], deng[:])
                wse = pb.tile([128, KC], F32, tag="W")
                exm_b = exm[:].rearrange(
                    "p (k h) -> p k h", h=H)[:, :, :, None].to_broadcast(
                    [128, Dt, H, HD])
                nc.vector.tensor_tensor(
                    out=wse[:].rearrange("p (k h d) -> p k h d", h=H, d=HD),
                    in0=se[:].rearrange("p (k h d) -> p k h d", h=H, d=HD),
                    in1=exm_b, op=ALU.mult)
                num = spool.tile([128, F], F32, tag="num")
                nc.vector.tensor_reduce(
                    out=num[:],
                    in_=wse[:].rearrange("p (k c) -> p c k", c=F),
                    axis=mybir.AxisListType.X, op=ALU.add)
                ot = spool.tile([128, F], F32, tag="ot")
                rec_b = rec[:][:, :, None].to_broadcast([128, H, HD])
                nc.vector.tensor_tensor(
                    out=ot[:].rearrange("p (h d) -> p h d", h=H),
                    in0=num[:].rearrange("p (h d) -> p h d", h=H),
                    in1=rec_b, op=ALU.mult)
                nc.sync.dma_start(out=outp[t * 128:(t + 1) * 128, :], in_=ot[:])

            for rep in range(P2REPS if n_main else 0):
                for t in range(n_main):
                    Dt = D_t[t]
                    num_idx = 128 * Dt + 1
                    ncol = 8 * Dt + 1
                    if VARIANT == "compute_only":
                        ch = chc
                    else:
                        ch = pch.tile([128, (Dt + 1) * 2 * F], F16, tag="ch",
                                      name=f"ch{t}r{rep}")
                        nc.gpsimd.dma_gather(
                            ch[:].rearrange("p (k c) -> p k c", c=2 * F),
                            table[BIAS:, :],
                            idx_sb[:, O16[t]:O16[t] + ncol],
                            num_idx, num_idx, 2 * F,
                            single_packet=False, queue_num=t % 4,
                        )
                    compute_tile(t, ch, rep)

    return nc


def kernel(x, Ws, bs, Wr, br, aw, ab, senders, receivers):
    x = np.asarray(x, np.float32)
    senders = np.asarray(senders, np.int32)
    receivers = np.asarray(receivers, np.int32)
    ins, meta = _host_prep(x, np.asarray(Ws), np.asarray(bs), np.asarray(Wr),
                           np.asarray(br), np.asarray(aw), np.asarray(ab),
                           senders, receivers)
    nc = _build_program(meta)
    if not nc.is_finalized():
        nc.finalize()
    in_maps = []
    for c in range(NC_CORES):
        in_maps.append({
            "xT": ins["xT"],
            "xlT": ins["xlT"][c],
            "Wsb": ins["Wsb"],
            "Wrb": ins["Wrb"],
            "awb": ins["awb"],
            "idx16": ins["idx16"][c],
            "b0": ins["b0"][c],
            "mask": ins["mask"][c],
        })
    res = run_bass_kernel_spmd(nc, in_maps, core_ids=list(range(NC_CORES)))
    N = x.shape[0]
    order = meta["order"]
    out_full = np.zeros((N, F), dtype=np.float32)
    for c in range(NC_CORES):
        rows = order[c::NC_CORES]
        out_full[rows] = res.results[c]["out"][: len(rows)]
    return out_full
